# revision 6
# baseline (speedup 1.0000x reference)
"""Trainium2 Bass kernel for masked multi-head attention.

Problem (hardcoded): B=2, S=2048, H=16, D_head=64, D_IN=OUT_DIM=1024, fp32 I/O.

Sharding: 8 cores = 2 (batch) x 4 (head-groups of 4 heads). Each core gets its
batch's q/k/v (pre-transposed to [D_IN, S] and cast to bf16 on the host) and
its head-group's weight columns. Each core computes its [S, 256] slice of the
output; the host reassembles the full [B, S, 1024] tensor. No collectives.

Device dataflow (all-transposed layout; no on-chip transposes):
  qwT/kwT = Wg^T @ xT              [256, S] as two [128, S] m-tiles, bf16
  vw      = v @ Wv_g               [S, 256] natural layout
  scoresT = kw_h @ qw_h^T          [S_k, S_q] psum tiles, PE K=64
  expT    = exp(scoresT + mask)    ScalarE, bias = -30000 on masked keys
  U^T & D = [vw_h | ones]^T @ expT col-packed PE accumulation over k
            (psum rows 0..63 = numerator^T, rows 64..127 = softmax denom)
  out     = U^T * qmask / D        VectorE recip + SBUF->SBUF shift DMA
"""

import sys
import numpy as np

sys.path.insert(0, "/opt/trn_rl_repo")

import ml_dtypes

BF16 = np.dtype(ml_dtypes.bfloat16)

B = 2
S = 2048
H = 16
DH = 64
D_IN = 1024
OUT_DIM = 1024
N_CORES = 8
HEADS_PER_CORE = 4
MCOLS = HEADS_PER_CORE * DH  # 256
MASK_NEG = -30000.0


def build_nc(s=S):
    """Build the single-core Bass graph (SPMD: same graph on all 8 cores)."""
    import concourse.bass as bass
    import concourse.bacc as bacc
    import concourse.tile as tile
    from concourse import mybir
    from contextlib import ExitStack

    f32 = mybir.dt.float32
    bf16 = mybir.dt.bfloat16

    nkt = s // 128          # scoresT partition tiles along k
    nqb = s // 512          # q blocks of 512
    nch = D_IN // 128       # d_in chunks

    nc = bacc.Bacc("TRN2", target_bir_lowering=False, debug=False,
                   num_devices=N_CORES)

    qT_ext = nc.dram_tensor("qT", [D_IN, s], bf16, kind="ExternalInput").ap()
    kT_ext = nc.dram_tensor("kT", [D_IN, s], bf16, kind="ExternalInput").ap()
    vT_ext = nc.dram_tensor("vT", [D_IN, s], bf16, kind="ExternalInput").ap()
    wq_ext = nc.dram_tensor("wq", [D_IN, MCOLS], bf16, kind="ExternalInput").ap()
    wk_ext = nc.dram_tensor("wk", [D_IN, MCOLS], bf16, kind="ExternalInput").ap()
    wv_ext = nc.dram_tensor("wv", [D_IN, MCOLS], bf16, kind="ExternalInput").ap()
    mb_ext = nc.dram_tensor("mb", [128, nkt], f32, kind="ExternalInput").ap()
    qm_ext = nc.dram_tensor("qm", [1, s], f32, kind="ExternalInput").ap()
    out_ext = nc.dram_tensor("out", [MCOLS, s], f32, kind="ExternalOutput").ap()

    Exp = mybir.ActivationFunctionType.Exp

    with tile.TileContext(nc) as tc:
        with ExitStack() as ctx:
            wpool = ctx.enter_context(tc.tile_pool(name="wpool", bufs=1))
            xpool = ctx.enter_context(tc.tile_pool(name="xpool", bufs=3))
            qkw = ctx.enter_context(tc.tile_pool(name="qkw", bufs=1))
            vwp = ctx.enter_context(tc.tile_pool(name="vwp", bufs=1))
            expp = ctx.enter_context(tc.tile_pool(name="expp", bufs=2))
            scp = ctx.enter_context(tc.tile_pool(name="scp", bufs=1))
            outp = ctx.enter_context(tc.tile_pool(name="outp", bufs=2))
            misc = ctx.enter_context(tc.tile_pool(name="misc", bufs=1))
            psS = ctx.enter_context(tc.tile_pool(name="psS", bufs=1, space="PSUM"))
            psA = ctx.enter_context(tc.tile_pool(name="psA", bufs=1, space="PSUM"))

            # ---- constants / small inputs ----
            mb_sb = misc.tile([128, nkt], f32)          # additive key-mask bias
            nc.sync.dma_start(out=mb_sb[:], in_=mb_ext[:])
            qm_bc = misc.tile([128, s], f32)            # qmask bcast to all parts
            qm_ap = qm_ext[:]
            qm_bcast_src = bass.AP(tensor=qm_ap.tensor, offset=qm_ap.offset,
                                   ap=[[0, 128]] + qm_ap.ap[1:])
            nc.sync.dma_start(out=qm_bc[:], in_=qm_bcast_src)
            ones64 = misc.tile([128, DH], bf16)         # lhsT for denominator
            nc.vector.memset(ones64[:], 1.0)

            # ---- weights: [D_IN, 256] -> [128, nch, 256] ----
            w_sb = {}
            for wnm, ext in (("wq", wq_ext), ("wk", wk_ext), ("wv", wv_ext)):
                wt = wpool.tile([128, nch, MCOLS], bf16, name=wnm, tag=wnm)
                nc.sync.dma_start(
                    out=wt[:],
                    in_=ext.rearrange("(c p) m -> p c m", p=128))
                w_sb[wnm] = wt

            # ---- q/k projections -> qwT/kwT [128, 2, s] bf16 ----
            # (head h lives at partitions 64*(h%2) .. +64 of m-tile h//2)
            qwT = qkw.tile([128, 2, s], bf16)
            kwT = qkw.tile([128, 2, s], bf16)
            for xext, wnm, dst in ((qT_ext, "wq", qwT), (kT_ext, "wk", kwT)):
                for mt in range(2):
                    pP = psS.tile([128, nqb * 512], f32, tag="psS", name="pP")
                    for c in range(nch):
                        xc = xpool.tile([128, s], bf16, tag="xstream", name="xc")
                        nc.sync.dma_start(
                            out=xc[:], in_=xext[c * 128:(c + 1) * 128, :])
                        for qb in range(nqb):
                            nc.tensor.matmul(
                                pP[:, qb * 512:(qb + 1) * 512],
                                w_sb[wnm][:, c, mt * 128:(mt + 1) * 128],
                                xc[:, qb * 512:(qb + 1) * 512],
                                start=(c == 0), stop=(c == nch - 1))
                    nc.vector.tensor_copy(dst[:, mt, :], pP[:, :])

            # ---- v projection -> vw [128, nkt, 256] bf16 (natural layout) ----
            # one accumulation group per PSUM bank: 4 s-tiles/round in the
            # four 1-bank psA slots (vT is re-streamed per round)
            vw = vwp.tile([128, nkt, MCOLS], bf16)
            n_vst = 4
            for r0 in range(0, nkt, n_vst):
                cnt = min(n_vst, nkt - r0)
                pVs = [psA.tile([128, 512], f32, tag=f"psA{j}", name="pV")
                       for j in range(cnt)]
                for c in range(nch):
                    vc = xpool.tile([128, s], bf16, tag="xstream", name="vc")
                    nc.sync.dma_start(out=vc[:], in_=vT_ext[c * 128:(c + 1) * 128, :])
                    for st in range(cnt):
                        nc.tensor.matmul(
                            pVs[st][:, 0:MCOLS],
                            vc[:, (r0 + st) * 128:(r0 + st + 1) * 128],
                            w_sb["wv"][:, c, :],
                            start=(c == 0), stop=(c == nch - 1))
                for st in range(cnt):
                    nc.vector.tensor_copy(vw[:, r0 + st, :], pVs[st][:, 0:MCOLS])

            # ---- attention, head by head ----
            for h in range(HEADS_PER_CORE):
                hp = 64 * (h % 2)
                mt = h // 2

                expT = expp.tile([128, nkt, s], bf16, tag="expT", name="expT")

                # S-phase: scoresT tiles + exp
                for kt in range(nkt):
                    ps = psS.tile([128, s], f32, tag="psS", name="ps")
                    for qb in range(nqb):
                        nc.tensor.matmul(
                            ps[:, qb * 512:(qb + 1) * 512],
                            kwT[hp:hp + 64, mt, kt * 128:(kt + 1) * 128],
                            qwT[hp:hp + 64, mt, qb * 512:(qb + 1) * 512],
                            start=True, stop=True)
                    nc.scalar.activation(expT[:, kt, :], ps[:, :], Exp,
                                         bias=mb_sb[:, kt:kt + 1], scale=1.0)

                # AV-phase, two q-blocks at a time (4 psum banks):
                #   bank psA{2j}:   U^T rows 0..63   (array cols 0..63)
                #   bank psA{2j+1}: denom rows 64..127 (array cols 64..127)
                # The U and denominator matmuls col-pack in the PE array.
                # NB: reciprocal_approx_fast is wrong at base_partition != 0 on
                # HW, so denominators are staged to partitions 0..63 first
                # (cross-base tensor_copy is fine).
                sc = scp.tile([64, s], f32, tag="sc", name="sc")
                den = scp.tile([64, s], f32, tag="den", name="den")
                for qbp in range(0, nqb, 2):
                    npair = min(2, nqb - qbp)
                    pUs = []
                    for j in range(npair):
                        qb = qbp + j
                        pU = psA.tile([128, 512], f32, tag=f"psA{2 * j}",
                                      name="pU")
                        pD = psA.tile([128, 512], f32, tag=f"psA{2 * j + 1}",
                                      name="pD")
                        pUs.append(pU)
                        for c in range(nkt):
                            rhs = expT[:, c, qb * 512:(qb + 1) * 512]
                            nc.tensor.matmul(
                                pU[0:64, :],
                                vw[:, c, h * DH:(h + 1) * DH],
                                rhs, start=(c == 0), stop=(c == nkt - 1),
                                tile_position=(0, 0))
                            nc.tensor.matmul(
                                pD[64:128, :],
                                ones64[:, :],
                                rhs, start=(c == 0), stop=(c == nkt - 1),
                                tile_position=(0, 64))
                        nc.vector.tensor_copy(
                            den[:, qb * 512:(qb + 1) * 512], pD[64:128, :])
                    lo = qbp * 512
                    hi = (qbp + npair) * 512
                    nc.vector.reciprocal_approx_fast(sc[:, lo:hi], den[:, lo:hi])
                    nc.vector.tensor_mul(sc[:, lo:hi], sc[:, lo:hi],
                                         qm_bc[0:64, lo:hi])
                    for j in range(npair):
                        qb = qbp + j
                        ot = outp.tile([64, 512], f32, tag="osb", name="ot")
                        nc.vector.tensor_mul(ot[:], pUs[j][0:64, :],
                                             sc[:, qb * 512:(qb + 1) * 512])
                        nc.sync.dma_start(
                            out=out_ext[h * DH:(h + 1) * DH,
                                        qb * 512:(qb + 1) * 512],
                            in_=ot[:])

    nc.compile()
    return nc


def shard_inputs(q, k, v, v_mask, q_mask, Wq, Wk, Wv, s=S):
    """Host-side sharding: core i -> (batch i//4, head-group i%4)."""
    scale = np.float32(1.0 / np.sqrt(DH))
    nkt = s // 128
    in_maps = []
    qT = [np.ascontiguousarray(np.asarray(q)[b, :s].T).astype(BF16) for b in range(B)]
    kT = [np.ascontiguousarray(np.asarray(k)[b, :s].T).astype(BF16) for b in range(B)]
    vT = [np.ascontiguousarray(np.asarray(v)[b, :s].T).astype(BF16) for b in range(B)]
    mb = []
    qm = []
    for b in range(B):
        bias = np.where(np.asarray(v_mask)[b, :s, 0] > 0.5, 0.0,
                        MASK_NEG).astype(np.float32)
        mb.append(np.ascontiguousarray(bias.reshape(nkt, 128).T))  # [128, nkt]
        qm.append(np.ascontiguousarray(
            np.asarray(q_mask)[b, :s, 0].reshape(1, s).astype(np.float32)))
    Wq = np.asarray(Wq)
    Wk = np.asarray(Wk)
    Wv = np.asarray(Wv)
    for i in range(N_CORES):
        b, g = divmod(i, HEADS_PER_CORE)
        cols = slice(g * MCOLS, (g + 1) * MCOLS)
        in_maps.append({
            "qT": qT[b],
            "kT": kT[b],
            "vT": vT[b],
            "wq": np.ascontiguousarray(Wq[:, cols] * scale).astype(BF16),
            "wk": np.ascontiguousarray(Wk[:, cols]).astype(BF16),
            "wv": np.ascontiguousarray(Wv[:, cols]).astype(BF16),
            "mb": mb[b],
            "qm": qm[b],
        })
    return in_maps


_CACHED = {}


def _get_compiled(s=S):
    if s not in _CACHED:
        _CACHED[s] = build_nc(s)
    return _CACHED[s]


def kernel(q, k, v, v_mask, q_mask, Wq, Wk, Wv):
    from concourse.bass_utils import run_bass_kernel_spmd

    nc = _get_compiled(S)
    in_maps = shard_inputs(q, k, v, v_mask, q_mask, Wq, Wk, Wv, S)
    res = run_bass_kernel_spmd(nc, in_maps, core_ids=list(range(N_CORES)))
    out = np.empty((B, S, OUT_DIM), dtype=np.float32)
    for i in range(N_CORES):
        b, g = divmod(i, HEADS_PER_CORE)
        out[b, :, g * MCOLS:(g + 1) * MCOLS] = res.results[i]["out"].T
    return out


# revision 8
# speedup vs baseline: 1.2969x; 1.2969x over previous
"""Trainium2 Bass kernel for masked multi-head attention.

Problem (hardcoded): B=2, S=2048, H=16, D_head=64, D_IN=OUT_DIM=1024, fp32 I/O.

Sharding: 8 cores = 2 (batch) x 4 (head-groups of 4 heads). Each core gets its
batch's q/k/v (pre-transposed to [D_IN, S] and cast to bf16 on the host) and
its head-group's weight columns. Each core computes its [S, 256] slice of the
output; the host reassembles the full [B, S, 1024] tensor. No collectives.

Device dataflow (all-transposed layout; no on-chip transposes), flash-style:
  qwT/kwT = Wg^T @ xT              [256, S] as two [128, S] m-tiles, bf16
  vw      = v @ Wv_g               [S, 256] natural layout
  per head, per k-tile kt (streamed, expT NOT materialized for all kt):
    scoresT(kt) = kw_h @ qw_h^T    [128, S] psum, PE K=64, [128,1024] tiles
    expT(kt)    = exp(scoresT + mask)  ScalarE, bias=-30000 on masked keys
    U^T & D    += [vw_h | ones]^T @ expT(kt)   PE col-packed (0,0)/(0,64)
                  into one PSUM bank per q-block; accumulation group stays
                  open across all kt (U starts the bank, D element-merges)
  out     = U^T * qmask / D        VectorE (recip at base partition 0)
"""

import sys
import numpy as np

sys.path.insert(0, "/opt/trn_rl_repo")

import ml_dtypes

BF16 = np.dtype(ml_dtypes.bfloat16)

B = 2
S = 2048
H = 16
DH = 64
D_IN = 1024
OUT_DIM = 1024
N_CORES = 8
HEADS_PER_CORE = 4
MCOLS = HEADS_PER_CORE * DH  # 256
MASK_NEG = -30000.0


def build_nc(s=S):
    """Build the single-core Bass graph (SPMD: same graph on all 8 cores)."""
    import concourse.bass as bass
    import concourse.bacc as bacc
    import concourse.tile as tile
    from concourse import mybir
    from contextlib import ExitStack

    f32 = mybir.dt.float32
    bf16 = mybir.dt.bfloat16

    nkt = s // 128          # scoresT partition tiles along k
    nqb = s // 512          # q blocks of 512
    nch = D_IN // 128       # d_in chunks
    qh_w = min(2, nqb)      # q-blocks per S psum tile ([128, 1024] normally)
    nqh = nqb // qh_w

    nc = bacc.Bacc("TRN2", target_bir_lowering=False, debug=False,
                   num_devices=N_CORES)

    qT_ext = nc.dram_tensor("qT", [D_IN, s], bf16, kind="ExternalInput").ap()
    kT_ext = nc.dram_tensor("kT", [D_IN, s], bf16, kind="ExternalInput").ap()
    vT_ext = nc.dram_tensor("vT", [D_IN, s], bf16, kind="ExternalInput").ap()
    wq_ext = nc.dram_tensor("wq", [D_IN, MCOLS], bf16, kind="ExternalInput").ap()
    wk_ext = nc.dram_tensor("wk", [D_IN, MCOLS], bf16, kind="ExternalInput").ap()
    wv_ext = nc.dram_tensor("wv", [D_IN, MCOLS], bf16, kind="ExternalInput").ap()
    mb_ext = nc.dram_tensor("mb", [128, nkt], f32, kind="ExternalInput").ap()
    qm_ext = nc.dram_tensor("qm", [1, s], f32, kind="ExternalInput").ap()
    out_ext = nc.dram_tensor("out", [MCOLS, s], f32, kind="ExternalOutput").ap()

    Exp = mybir.ActivationFunctionType.Exp

    with tile.TileContext(nc) as tc:
        with ExitStack() as ctx:
            wpool = ctx.enter_context(tc.tile_pool(name="wpool", bufs=1))
            xpool = ctx.enter_context(tc.tile_pool(name="xpool", bufs=1))
            qkw = ctx.enter_context(tc.tile_pool(name="qkw", bufs=1))
            vwp = ctx.enter_context(tc.tile_pool(name="vwp", bufs=1))
            expp = ctx.enter_context(tc.tile_pool(name="expp", bufs=3))
            scp = ctx.enter_context(tc.tile_pool(name="scp", bufs=1))
            outp = ctx.enter_context(tc.tile_pool(name="outp", bufs=2))
            misc = ctx.enter_context(tc.tile_pool(name="misc", bufs=1))
            psS = ctx.enter_context(tc.tile_pool(name="psS", bufs=2, space="PSUM"))
            psA = ctx.enter_context(tc.tile_pool(name="psA", bufs=1, space="PSUM"))

            # ---- constants / small inputs ----
            mb_sb = misc.tile([128, nkt], f32)          # additive key-mask bias
            nc.sync.dma_start(out=mb_sb[:], in_=mb_ext[:])
            qm_bc = misc.tile([64, s], f32)             # qmask bcast, rows 0..63
            qm_ap = qm_ext[:]
            qm_bcast_src = bass.AP(tensor=qm_ap.tensor, offset=qm_ap.offset,
                                   ap=[[0, 64]] + qm_ap.ap[1:])
            nc.sync.dma_start(out=qm_bc[:], in_=qm_bcast_src)
            ones64 = misc.tile([128, DH], bf16)         # lhsT for denominator
            nc.vector.memset(ones64[:], 1.0)

            # ---- weights: [D_IN, 256] -> [128, nch, 256] ----
            w_sb = {}
            for wnm, ext in (("wq", wq_ext), ("wk", wk_ext), ("wv", wv_ext)):
                wt = wpool.tile([128, nch, MCOLS], bf16, name=wnm, tag=wnm)
                nc.sync.dma_start(
                    out=wt[:],
                    in_=ext.rearrange("(c p) m -> p c m", p=128))
                w_sb[wnm] = wt

            # ---- resident activations: [128, nch, s] bf16, one DMA each ----
            x_sb = {}
            for xnm, ext in (("q", qT_ext), ("k", kT_ext), ("v", vT_ext)):
                xt = xpool.tile([128, nch, s], bf16, name="x" + xnm, tag="x" + xnm)
                nc.sync.dma_start(
                    out=xt[:], in_=ext.rearrange("(c p) n -> p c n", p=128))
                x_sb[xnm] = xt

            # ---- q/k projections -> qwT/kwT [128, 2, s] bf16 ----
            # (head h lives at partitions 64*(h%2) .. +64 of m-tile h//2)
            qwT = qkw.tile([128, 2, s], bf16)
            kwT = qkw.tile([128, 2, s], bf16)
            for xnm, wnm, dst in (("q", "wq", qwT), ("k", "wk", kwT)):
                for mt in range(2):
                    pPs = [psA.tile([128, 512], f32, tag=f"psA{j}", name="pP")
                           for j in range(nqb)]
                    for c in range(nch):
                        for qb in range(nqb):
                            nc.tensor.matmul(
                                pPs[qb][:, :],
                                w_sb[wnm][:, c, mt * 128:(mt + 1) * 128],
                                x_sb[xnm][:, c, qb * 512:(qb + 1) * 512],
                                start=(c == 0), stop=(c == nch - 1))
                    for qb in range(nqb):
                        nc.vector.tensor_copy(
                            dst[:, mt, qb * 512:(qb + 1) * 512], pPs[qb][:, :])

            # ---- v projection -> vw [128, nkt, 256] bf16 (natural layout) ----
            vw = vwp.tile([128, nkt, MCOLS], bf16)
            n_vst = 4
            for r0 in range(0, nkt, n_vst):
                cnt = min(n_vst, nkt - r0)
                pVs = [psA.tile([128, 512], f32, tag=f"psA{j}", name="pV")
                       for j in range(cnt)]
                for c in range(nch):
                    for st in range(cnt):
                        nc.tensor.matmul(
                            pVs[st][:, 0:MCOLS],
                            x_sb["v"][:, c, (r0 + st) * 128:(r0 + st + 1) * 128],
                            w_sb["wv"][:, c, :],
                            start=(c == 0), stop=(c == nch - 1))
                for st in range(cnt):
                    nc.vector.tensor_copy(vw[:, r0 + st, :], pVs[st][:, 0:MCOLS])

            # ---- attention, head by head (flash-style over k tiles) ----
            for h in range(HEADS_PER_CORE):
                hp = 64 * (h % 2)
                mt = h // 2

                # per-q-block U/D accumulators, one PSUM bank each, held
                # open across the whole k loop
                pQ = [psA.tile([128, 512], f32, tag=f"psA{qb}", name="pQ")
                      for qb in range(nqb)]

                def av_burst(kt, et):
                    for qb in range(nqb):
                        rhs = et[:, qb * 512:(qb + 1) * 512]
                        # Two col-packed accumulation groups share the bank;
                        # HW-verified: start=True clears has_written only for
                        # the addressed partitions (psum_probe.py), so each
                        # group starts itself. skip_group_check silences the
                        # simulator's partition-agnostic zero-region check.
                        nc.tensor.matmul(
                            pQ[qb][0:64, :],
                            vw[:, kt, h * DH:(h + 1) * DH],
                            rhs, start=(kt == 0), stop=(kt == nkt - 1),
                            tile_position=(0, 0), skip_group_check=True)
                        nc.tensor.matmul(
                            pQ[qb][64:128, :],
                            ones64[:, :],
                            rhs, start=(kt == 0), stop=(kt == nkt - 1),
                            tile_position=(0, 64), skip_group_check=True)

                prev = None  # (kt, expT tile)
                for kt in range(nkt):
                    expT = expp.tile([128, s], bf16, tag="expT", name="expT")
                    for qh in range(nqh):
                        ps = psS.tile([128, qh_w * 512], f32, tag="psS",
                                      name="ps")
                        for j in range(qh_w):
                            qb = qh * qh_w + j
                            nc.tensor.matmul(
                                ps[:, j * 512:(j + 1) * 512],
                                kwT[hp:hp + 64, mt, kt * 128:(kt + 1) * 128],
                                qwT[hp:hp + 64, mt, qb * 512:(qb + 1) * 512],
                                start=True, stop=True)
                        nc.scalar.activation(
                            expT[:, qh * qh_w * 512:(qh + 1) * qh_w * 512],
                            ps[:, :], Exp, bias=mb_sb[:, kt:kt + 1], scale=1.0)
                    if prev is not None:
                        av_burst(*prev)
                    prev = (kt, expT)
                av_burst(*prev)

                # ---- normalization ----
                # (reciprocal_approx_fast is wrong at base_partition != 0 on
                # HW, so denominators are staged to partitions 0..63 first;
                # cross-base tensor_copy is fine)
                sc = scp.tile([64, s], f32, tag="sc", name="sc")
                den = scp.tile([64, s], f32, tag="den", name="den")
                for qb in range(nqb):
                    nc.vector.tensor_copy(den[:, qb * 512:(qb + 1) * 512],
                                          pQ[qb][64:128, :])
                nc.vector.reciprocal_approx_fast(sc[:, :], den[:, :])
                nc.vector.tensor_mul(sc[:, :], sc[:, :], qm_bc[:, :])
                for qb in range(nqb):
                    ot = outp.tile([64, 512], f32, tag="osb", name="ot")
                    nc.vector.tensor_mul(ot[:], pQ[qb][0:64, :],
                                         sc[:, qb * 512:(qb + 1) * 512])
                    nc.sync.dma_start(
                        out=out_ext[h * DH:(h + 1) * DH,
                                    qb * 512:(qb + 1) * 512],
                        in_=ot[:])

    nc.compile()
    return nc


def shard_inputs(q, k, v, v_mask, q_mask, Wq, Wk, Wv, s=S):
    """Host-side sharding: core i -> (batch i//4, head-group i%4)."""
    scale = np.float32(1.0 / np.sqrt(DH))
    nkt = s // 128
    in_maps = []
    qT = [np.ascontiguousarray(np.asarray(q)[b, :s].T).astype(BF16) for b in range(B)]
    kT = [np.ascontiguousarray(np.asarray(k)[b, :s].T).astype(BF16) for b in range(B)]
    vT = [np.ascontiguousarray(np.asarray(v)[b, :s].T).astype(BF16) for b in range(B)]
    mb = []
    qm = []
    for b in range(B):
        bias = np.where(np.asarray(v_mask)[b, :s, 0] > 0.5, 0.0,
                        MASK_NEG).astype(np.float32)
        mb.append(np.ascontiguousarray(bias.reshape(nkt, 128).T))  # [128, nkt]
        qm.append(np.ascontiguousarray(
            np.asarray(q_mask)[b, :s, 0].reshape(1, s).astype(np.float32)))
    Wq = np.asarray(Wq)
    Wk = np.asarray(Wk)
    Wv = np.asarray(Wv)
    for i in range(N_CORES):
        b, g = divmod(i, HEADS_PER_CORE)
        cols = slice(g * MCOLS, (g + 1) * MCOLS)
        in_maps.append({
            "qT": qT[b],
            "kT": kT[b],
            "vT": vT[b],
            "wq": np.ascontiguousarray(Wq[:, cols] * scale).astype(BF16),
            "wk": np.ascontiguousarray(Wk[:, cols]).astype(BF16),
            "wv": np.ascontiguousarray(Wv[:, cols]).astype(BF16),
            "mb": mb[b],
            "qm": qm[b],
        })
    return in_maps


_CACHED = {}


def _get_compiled(s=S):
    if s not in _CACHED:
        _CACHED[s] = build_nc(s)
    return _CACHED[s]


def kernel(q, k, v, v_mask, q_mask, Wq, Wk, Wv):
    from concourse.bass_utils import run_bass_kernel_spmd

    nc = _get_compiled(S)
    in_maps = shard_inputs(q, k, v, v_mask, q_mask, Wq, Wk, Wv, S)
    res = run_bass_kernel_spmd(nc, in_maps, core_ids=list(range(N_CORES)))
    out = np.empty((B, S, OUT_DIM), dtype=np.float32)
    for i in range(N_CORES):
        b, g = divmod(i, HEADS_PER_CORE)
        out[b, :, g * MCOLS:(g + 1) * MCOLS] = res.results[i]["out"].T
    return out
